# revision 5
# baseline (speedup 1.0000x reference)
"""AreaAttentionBlock Trainium2 kernel (8 NeuronCores, data-parallel).

Problem: B=2, C=256, H=W=64, HEADS=8 (hd=32), AREA=4, MLP_DIM=307.
The area split makes attention independent per (batch, area) group:
8 groups of 1024 pixels (16 image rows) -> one group per core.
Only cross-slab dependency is the 1-row halo of the depthwise 3x3 conv,
which the host pre-supplies in each core's x slab (zero-padded at image
top/bottom edges, matching the reference's zero conv padding).

Per-core pipeline (all matmuls bf16, fp32 PSUM accumulation):
  x -> [QKV 1x1 convs] -> q,k (ch-major) + v^T (px-major) + v4 (ch-major)
  v4 -> depthwise 3x3 via 9 diagonal-stationary matmuls (PSUM accumulate)
  attention: S^T[m,n] = K^T Q (4-way row-tiled K=32 matmuls)
             P^T = exp(scale*S^T) on ACT (no max subtraction: |S|<1)
             colsum_h[n] = sum_m P^T (all-ones col-tiled matmul)
             out = V^T.T @ P^T (col-tiled), normalize by 1/colsum
  proj 1x1, residual, MLP (silu via tanh: stays in exp ACT table set)
"""

import numpy as np
import ml_dtypes

C = 256
HEADS = 8
HD = 32
AREA = 4
MLP = 307
MLP_PAD = 384  # 3 k-tiles of 128
B, H, W = 2, 64, 64
NPX = 1024          # pixels per slab (16 rows)
NHALO = 1152        # 18 rows with halo
SCALE = float(1.0 / np.sqrt(HD))

BF16 = ml_dtypes.bfloat16

_COMPILED = {}


def _build_graph():
    import concourse.bass as bass
    import concourse.bacc as bacc
    import concourse.mybir as mybir
    import concourse.tile as tile

    f32 = mybir.dt.float32
    bf16 = mybir.dt.bfloat16
    AF = mybir.ActivationFunctionType
    OP = mybir.AluOpType

    nc = bacc.Bacc(target_bir_lowering=False)

    # ---- DRAM I/O ----
    xf_d = nc.dram_tensor("xf", [2, 128, NPX], f32, kind="ExternalInput")
    xb_d = nc.dram_tensor("xb", [2, 128, NHALO], bf16, kind="ExternalInput")
    wqkv_d = nc.dram_tensor("wqkv", [2, 128, 768], bf16, kind="ExternalInput")
    wproj_d = nc.dram_tensor("wproj", [2, 128, 256], bf16, kind="ExternalInput")
    wm1_d = nc.dram_tensor("wm1", [2, 128, MLP], bf16, kind="ExternalInput")
    wm2_d = nc.dram_tensor("wm2", [3, 128, 256], bf16, kind="ExternalInput")
    dwdiag_d = nc.dram_tensor("dwdiag", [2, 128, 9, 128], bf16, kind="ExternalInput")
    bvrow_d = nc.dram_tensor("bvrow", [1, 256], bf16, kind="ExternalInput")
    bq_d = nc.dram_tensor("bq", [2, 128, 1], f32, kind="ExternalInput")
    bk_d = nc.dram_tensor("bk", [2, 128, 1], f32, kind="ExternalInput")
    bv_d = nc.dram_tensor("bv", [2, 128, 1], f32, kind="ExternalInput")
    bproj_d = nc.dram_tensor("bproj", [2, 128, 1], f32, kind="ExternalInput")
    bm1_d = nc.dram_tensor("bm1", [3, 128, 1], f32, kind="ExternalInput")
    bm1h_d = nc.dram_tensor("bm1h", [3, 128, 1], f32, kind="ExternalInput")
    bm2_d = nc.dram_tensor("bm2", [2, 128, 1], f32, kind="ExternalInput")
    out_d = nc.dram_tensor("out", [2, 128, NPX], f32, kind="ExternalOutput")

    with tile.TileContext(nc) as tc:
        with (
            tc.sbuf_pool(name="weights", bufs=1) as wp,
            tc.sbuf_pool(name="acts", bufs=1) as ap,
            tc.sbuf_pool(name="pt_pool", bufs=4) as ptp,
            tc.sbuf_pool(name="small", bufs=2) as sp,
        ):
            # ---- load weights / biases ----
            wqkv = [wp.tile([128, 768], bf16, tag=f"wqkv{k}", name=f"wqkv{k}") for k in range(2)]
            wproj = [wp.tile([128, 256], bf16, tag=f"wproj{k}", name=f"wproj{k}") for k in range(2)]
            wm1 = [wp.tile([128, MLP], bf16, tag=f"wm1{k}", name=f"wm1{k}") for k in range(2)]
            wm2 = [wp.tile([128, 256], bf16, tag=f"wm2{k}", name=f"wm2{k}") for k in range(3)]
            dwdiag = [wp.tile([128, 9, 128], bf16, tag=f"dw{k}", name=f"dw{k}") for k in range(2)]
            for k in range(2):
                nc.sync.dma_start(out=wqkv[k][:], in_=wqkv_d[k])
                nc.sync.dma_start(out=wproj[k][:], in_=wproj_d[k])
                nc.sync.dma_start(out=wm1[k][:], in_=wm1_d[k])
                nc.sync.dma_start(out=dwdiag[k][:], in_=dwdiag_d[k])
            for k in range(3):
                nc.sync.dma_start(out=wm2[k][:], in_=wm2_d[k])
            bvrow = wp.tile([1, 256], bf16, tag="bvrow", name="bvrow")
            nc.sync.dma_start(out=bvrow[:], in_=bvrow_d[:])
            bq = wp.tile([128, 2], f32, tag="bq2", name="bq2")
            bk = wp.tile([128, 2], f32, tag="bk2", name="bk2")
            bv = wp.tile([128, 2], f32, tag="bv2", name="bv2")
            bproj = wp.tile([128, 2], f32, tag="bproj2", name="bproj2")
            bm2 = wp.tile([128, 2], f32, tag="bm22", name="bm22")
            bm1 = wp.tile([128, 3], f32, tag="bm1", name="bm1")
            bm1h = wp.tile([128, 3], f32, tag="bm1h", name="bm1h")
            for k in range(2):
                nc.sync.dma_start(out=bq[:, k : k + 1], in_=bq_d[k])
                nc.sync.dma_start(out=bk[:, k : k + 1], in_=bk_d[k])
                nc.sync.dma_start(out=bv[:, k : k + 1], in_=bv_d[k])
                nc.sync.dma_start(out=bproj[:, k : k + 1], in_=bproj_d[k])
                nc.sync.dma_start(out=bm2[:, k : k + 1], in_=bm2_d[k])
            for k in range(3):
                nc.sync.dma_start(out=bm1[:, k : k + 1], in_=bm1_d[k])
                nc.sync.dma_start(out=bm1h[:, k : k + 1], in_=bm1h_d[k])

            ones32 = wp.tile([128, 32], bf16, tag="ones32", name="ones32")
            nc.vector.memset(ones32[:], 1.0)
            onesrow = wp.tile([1, 128], bf16, tag="onesrow", name="onesrow")
            nc.vector.memset(onesrow[:], 1.0)
            zrow = wp.tile([1, 128], bf16, tag="zrow", name="zrow")
            nc.vector.memset(zrow[:], 0.0)
            zrow512 = wp.tile([1, 512], bf16, tag="zrow512", name="zrow512")
            nc.vector.memset(zrow512[:], 0.0)

            # ---- load x ----
            xb = [ap.tile([128, NHALO], bf16, tag=f"xb{k}", name=f"xb{k}") for k in range(2)]
            xf = [ap.tile([128, NPX], f32, tag=f"xf{k}", name=f"xf{k}") for k in range(2)]
            for k in range(2):
                nc.sync.dma_start(out=xb[k][:], in_=xb_d[k])
                nc.sync.dma_start(out=xf[k][:], in_=xf_d[k])

            # ---- persistent activation tiles ----
            q_sb = [ap.tile([128, NPX], bf16, tag=f"q{g}", name=f"q{g}") for g in range(2)]
            k_sb = [ap.tile([128, NPX], bf16, tag=f"k{g}", name=f"k{g}") for g in range(2)]
            vT = [ap.tile([128, 256], bf16, tag=f"vT{p}", name=f"vT{p}") for p in range(8)]
            v4pad = [ap.tile([128, 18, 66], bf16, tag=f"v4p{g}", name=f"v4p{g}") for g in range(2)]
            pe_sb = [ap.tile([128, NPX], bf16, tag=f"pe{g}", name=f"pe{g}") for g in range(2)]
            attn = [ap.tile([128, NPX], bf16, tag=f"attn{g}", name=f"attn{g}") for g in range(2)]
            x1f = [ap.tile([128, NPX], f32, tag=f"x1f{g}", name=f"x1f{g}") for g in range(2)]
            x1b = [ap.tile([128, NPX], bf16, tag=f"x1b{g}", name=f"x1b{g}") for g in range(2)]
            u_sb = [ap.tile([128, NPX], bf16, tag=f"u{m}", name=f"u{m}") for m in range(3)]
            out_sb = [ap.tile([128, NPX], f32, tag=f"osb{g}", name=f"osb{g}") for g in range(2)]

            for g in range(2):
                nc.vector.memset(v4pad[g][:], 0.0)

            # ================= Phase A: QKV convs =================
            with tc.psum_pool(name="psA", bufs=2) as psA:
                # Q and K: out channels 0..255 (q), 256..511 (k)
                for which, dst, bias in ((0, q_sb, bq), (1, k_sb, bk)):
                    for g in range(2):
                        ps = psA.tile([128, NPX], f32, tag="qk", name="qk", bufs=1)
                        mt = 256 * which + 128 * g
                        for ncc in range(2):
                            for kt in range(2):
                                nc.tensor.matmul(
                                    ps[:, 512 * ncc : 512 * ncc + 512],
                                    lhsT=wqkv[kt][:, mt : mt + 128],
                                    rhs=xb[kt][:, 64 + 512 * ncc : 64 + 512 * ncc + 512],
                                    start=(kt == 0),
                                    stop=(kt == 1),
                                )
                        nc.vector.tensor_scalar_add(
                            out=dst[g][:], in0=ps[:], scalar1=bias[:, g : g + 1]
                        )
                # V^T: [px, c] via x as stationary (+ ones x bvrow for bias)
                for p in range(8):
                    ps = psA.tile([128, 256], f32, tag="vt", name="vt", bufs=2)
                    px0 = 64 + 128 * p
                    for kt in range(2):
                        nc.tensor.matmul(
                            ps[:],
                            lhsT=xb[kt][:, px0 : px0 + 128],
                            rhs=wqkv[kt][:, 512:768],
                            start=(kt == 0),
                            stop=False,
                            skip_group_check=True,
                        )
                    nc.tensor.matmul(
                        ps[:],
                        lhsT=onesrow[:],
                        rhs=bvrow[:],
                        start=False,
                        stop=True,
                        skip_group_check=True,
                    )
                    nc.vector.tensor_copy(out=vT[p][:], in_=ps[:])
                # v4 (channel-major, with halo) for the depthwise conv
                for g in range(2):
                    ps = psA.tile([128, NHALO], f32, tag="v4", name="v4", bufs=1)
                    for c0, cw in ((0, 512), (512, 512), (1024, 128)):
                        for kt in range(2):
                            nc.tensor.matmul(
                                ps[:, c0 : c0 + cw],
                                lhsT=wqkv[kt][:, 512 + 128 * g : 640 + 128 * g],
                                rhs=xb[kt][:, c0 : c0 + cw],
                                start=(kt == 0),
                                stop=(kt == 1),
                            )
                    # strided copy into zero-padded [18, 66] layout (+bias)
                    nc.vector.tensor_scalar_add(
                        out=v4pad[g][:, :, 1:65],
                        in0=ps[:].rearrange("p (r w) -> p r w", w=64),
                        scalar1=bv[:, g : g + 1],
                    )

            # ================= Phase A2: depthwise 3x3 =================
            with tc.psum_pool(name="psPE", bufs=2) as psPE:
                for g in range(2):
                    ps = psPE.tile([128, NPX], f32, tag="pe", name="pe")
                    for ncc in range(2):
                        t = 0
                        for dy in (-1, 0, 1):
                            for dx in (-1, 0, 1):
                                nc.tensor.matmul(
                                    ps[:, 512 * ncc : 512 * ncc + 512],
                                    lhsT=dwdiag[g][:, t, :],
                                    rhs=v4pad[g][
                                        :,
                                        1 + dy + 8 * ncc : 9 + dy + 8 * ncc,
                                        1 + dx : 65 + dx,
                                    ],
                                    start=(t == 0),
                                    stop=(t == 8),
                                )
                                t += 1
                    nc.vector.tensor_copy(out=pe_sb[g][:], in_=ps[:])

            # ================= Phase B: attention =================
            # + Phase C/D (proj + MLP) interleaved per 512-px chunk
            with (
                tc.psum_pool(name="psS", bufs=2) as psS,
                tc.psum_pool(name="psAcc", bufs=1) as psAcc,
                tc.psum_pool(name="psMLP", bufs=2) as psMLP,
            ):

                def attention_block(ncc, hg):
                    av = psAcc.tile([128, 512], f32, tag="av", name="av")
                    cs = psAcc.tile([128, 512], f32, tag="cs", name="cs")
                    # zero-fill via K=1 matmul: sets has_written for the
                    # whole bank so all subsequent accumulates are safe
                    # regardless of scheduling order.
                    nc.tensor.matmul(
                        av[:], lhsT=zrow[:], rhs=zrow512[:],
                        start=True, stop=False, skip_group_check=True,
                    )
                    nc.tensor.matmul(
                        cs[:], lhsT=zrow[:], rhs=zrow512[:],
                        start=True, stop=False, skip_group_check=True,
                    )

                    def scores(j):
                        tiles = []
                        for half in range(2):
                            s_ps = psS.tile([128, 1024], f32, tag="s", name="s")
                            for hl in range(2):
                                h = 2 * half + hl
                                nc.tensor.matmul(
                                    s_ps[:, 512 * hl : 512 * hl + 512],
                                    lhsT=k_sb[hg][32 * h : 32 * h + 32, 128 * j : 128 * j + 128],
                                    rhs=q_sb[hg][32 * h : 32 * h + 32, 512 * ncc : 512 * ncc + 512],
                                    start=True,
                                    stop=True,
                                    tile_position=(32 * h, 0),
                                    skip_group_check=True,
                                )
                            tiles.append(s_ps)
                        return tiles

                    def consume(j, s_tiles, last):
                        pts = []
                        for half in range(2):
                            pt = ptp.tile([128, 1024], bf16, tag="pt", name="pt")
                            nc.scalar.activation(
                                pt[:], s_tiles[half][:], AF.Exp, scale=SCALE
                            )
                            pts.append(pt)
                        for h in range(4):
                            half, sub = h // 2, h % 2
                            rhs = pts[half][:, 512 * sub : 512 * sub + 512]
                            nc.tensor.matmul(
                                av[32 * h : 32 * h + 32, :],
                                lhsT=vT[j][:, 128 * hg + 32 * h : 128 * hg + 32 * h + 32],
                                rhs=rhs,
                                start=False,
                                stop=(last and h == 3),
                                tile_position=(0, 32 * h),
                                skip_group_check=True,
                            )
                            nc.tensor.matmul(
                                cs[32 * h : 32 * h + 32, :],
                                lhsT=ones32[:],
                                rhs=rhs,
                                start=False,
                                stop=(last and h == 3),
                                tile_position=(0, 32 * h),
                                skip_group_check=True,
                            )

                    prev = scores(0)
                    for j in range(8):
                        cur = prev
                        if j < 7:
                            prev = scores(j + 1)
                        consume(j, cur, j == 7)

                    recip = sp.tile([128, 512], f32, tag="recip", name="recip")
                    nc.vector.reciprocal_approx_fast(out=recip[:], in_=cs[:])
                    t1 = sp.tile([128, 512], bf16, tag="t1", name="t1")
                    nc.vector.tensor_mul(t1[:], av[:], recip[:])
                    nc.vector.tensor_add(
                        attn[hg][:, 512 * ncc : 512 * ncc + 512],
                        t1[:],
                        pe_sb[hg][:, 512 * ncc : 512 * ncc + 512],
                    )

                def mlp_block(ncc):
                    s = slice(512 * ncc, 512 * ncc + 512)
                    # proj + residual -> x1
                    for g in range(2):
                        ps = psMLP.tile([128, 512], f32, tag="mlp", name="mlp")
                        for kt in range(2):
                            nc.tensor.matmul(
                                ps[:],
                                lhsT=wproj[kt][:, 128 * g : 128 * g + 128],
                                rhs=attn[kt][:, s],
                                start=(kt == 0),
                                stop=(kt == 1),
                            )
                        nc.vector.scalar_tensor_tensor(
                            out=x1f[g][:, s], in0=ps[:], scalar=bproj[:, g : g + 1],
                            in1=xf[g][:, s], op0=OP.add, op1=OP.add,
                        )
                        nc.vector.tensor_copy(out=x1b[g][:, s], in_=x1f[g][:, s])
                    # m1 + silu (u = 2*silu(z)/... : u=(tanh(z/2)+1)*z; 0.5 folded in wm2)
                    for m in range(3):
                        mp = 128 if m < 2 else MLP - 256
                        ps = psMLP.tile([128, 512], f32, tag="mlp", name="mlp")
                        for kt in range(2):
                            nc.tensor.matmul(
                                ps[:mp, :],
                                lhsT=wm1[kt][:, 128 * m : 128 * m + mp],
                                rhs=x1b[kt][:, s],
                                start=(kt == 0),
                                stop=(kt == 1),
                            )
                        th = sp.tile([128, 512], bf16, tag="tanh", name="tanh")
                        nc.scalar.activation(
                            th[:mp, :], ps[:mp, :], AF.Tanh,
                            bias=bm1h[:mp, m : m + 1], scale=0.5,
                        )
                        z = sp.tile([128, 512], bf16, tag="z", name="z")
                        nc.vector.tensor_scalar_add(
                            out=z[:mp, :], in0=ps[:mp, :], scalar1=bm1[:mp, m : m + 1]
                        )
                        nc.vector.scalar_tensor_tensor(
                            out=u_sb[m][:mp, s], in0=th[:mp, :], scalar=1.0,
                            in1=z[:mp, :], op0=OP.add, op1=OP.mult,
                        )
                    # m2 + residual -> out
                    for g in range(2):
                        ps = psMLP.tile([128, 512], f32, tag="mlp", name="mlp")
                        for kt in range(3):
                            kp = 128 if kt < 2 else MLP - 256
                            nc.tensor.matmul(
                                ps[:],
                                lhsT=wm2[kt][:kp, 128 * g : 128 * g + 128],
                                rhs=u_sb[kt][:kp, s],
                                start=(kt == 0),
                                stop=(kt == 2),
                            )
                        nc.vector.scalar_tensor_tensor(
                            out=out_sb[g][:, s], in0=ps[:], scalar=bm2[:, g : g + 1],
                            in1=x1f[g][:, s], op0=OP.add, op1=OP.add,
                        )

                attention_block(0, 0)
                attention_block(0, 1)
                attention_block(1, 0)
                mlp_block(0)
                attention_block(1, 1)
                mlp_block(1)

                for g in range(2):
                    nc.sync.dma_start(out=out_d[g], in_=out_sb[g][:])

    nc.compile()
    return nc


def _get_graph():
    if "nc" not in _COMPILED:
        _COMPILED["nc"] = _build_graph()
    return _COMPILED["nc"]


def _prep_inputs(x, w_qk, s_qk, b_qk, w_v, s_v, b_v, w_pe, s_pe, b_pe,
                 w_proj, s_proj, b_proj, w_m1, s_m1, b_m1, w_m2, s_m2, b_m2):
    f32 = np.float32
    x = np.asarray(x, f32)
    w_qk = np.asarray(w_qk, f32) * np.asarray(s_qk, f32)[:, None]
    w_v_e = np.asarray(w_v, f32) * np.asarray(s_v, f32)[:, None]
    w_pe_e = np.asarray(w_pe, f32)[:, 0] * np.asarray(s_pe, f32)[:, None, None]
    w_proj_e = np.asarray(w_proj, f32) * np.asarray(s_proj, f32)[:, None]
    w_m1_e = np.asarray(w_m1, f32) * np.asarray(s_m1, f32)[:, None]
    w_m2_e = 0.5 * np.asarray(w_m2, f32) * np.asarray(s_m2, f32)[:, None]

    wqkvT = np.concatenate([w_qk[:C].T, w_qk[C:].T, w_v_e.T], axis=1)  # [256,768]
    wprojT = w_proj_e.T  # [256, 256]
    wm1T = w_m1_e.T      # [256, 307]
    wm2T = np.zeros((MLP_PAD, C), f32)
    wm2T[:MLP] = w_m2_e.T

    # diagonal stationaries for the depthwise conv, per (ctile, tap)
    # host layout [ct, p(part), tap, 128] so the DMA is contiguous
    dwdiag = np.zeros((2, 128, 9, 128), f32)
    eye = np.eye(128, dtype=f32)
    for ct in range(2):
        t = 0
        for dy in range(3):
            for dx in range(3):
                w_t = w_pe_e[128 * ct : 128 * ct + 128, dy, dx]
                dwdiag[ct, :, t, :] = eye * w_t[:, None]
                t += 1

    b_qk = np.asarray(b_qk, f32)
    b_v = np.asarray(b_v, f32)
    b_pe = np.asarray(b_pe, f32)
    b_proj_eff = np.asarray(b_proj, f32) + w_proj_e @ b_pe
    b_m1 = np.asarray(b_m1, f32)
    b_m1_pad = np.zeros(MLP_PAD, f32)
    b_m1_pad[:MLP] = b_m1
    b_m2 = np.asarray(b_m2, f32)

    common = {
        "wqkv": wqkvT.reshape(2, 128, 768).astype(BF16),
        "wproj": wprojT.reshape(2, 128, 256).astype(BF16),
        "wm1": wm1T.reshape(2, 128, MLP).astype(BF16),
        "wm2": wm2T.reshape(3, 128, 256).astype(BF16),
        "dwdiag": dwdiag.astype(BF16),
        "bvrow": b_v.reshape(1, 256).astype(BF16),
        "bq": b_qk[:C].reshape(2, 128, 1).astype(f32),
        "bk": b_qk[C:].reshape(2, 128, 1).astype(f32),
        "bv": b_v.reshape(2, 128, 1).astype(f32),
        "bproj": b_proj_eff.reshape(2, 128, 1).astype(f32),
        "bm1": b_m1_pad.reshape(3, 128, 1).astype(f32),
        "bm1h": (0.5 * b_m1_pad).reshape(3, 128, 1).astype(f32),
        "bm2": b_m2.reshape(2, 128, 1).astype(f32),
    }

    in_maps = []
    for core in range(8):
        b, a = core // AREA, core % AREA
        xs = np.zeros((C, 18, W), f32)
        r0 = 16 * a - 1
        lo, hi = max(r0, 0), min(r0 + 18, H)
        xs[:, lo - r0 : lo - r0 + (hi - lo)] = x[b, :, lo:hi]
        m = dict(common)
        m["xb"] = xs.reshape(C, NHALO).reshape(2, 128, NHALO).astype(BF16)
        m["xf"] = (
            xs[:, 1:17].reshape(C, NPX).reshape(2, 128, NPX).astype(f32)
        )
        in_maps.append(m)
    return in_maps


def kernel(**inputs):
    from concourse.bass_utils import run_bass_kernel_spmd

    nc = _get_graph()
    in_maps = _prep_inputs(**inputs)
    res = run_bass_kernel_spmd(nc, in_maps, core_ids=list(range(8)))
    out = np.zeros((B, C, H, W), np.float32)
    for core in range(8):
        b, a = core // AREA, core % AREA
        o = np.asarray(res.results[core]["out"], np.float32).reshape(C, 16, W)
        out[b, :, 16 * a : 16 * a + 16, :] = o
    return out
